# revision 10
# baseline (speedup 1.0000x reference)
"""Trainium2 Bass kernel for nn_NodeAdder (GGNN propagation + node-type head).

Strategy (8 NeuronCores, SPMD, no collectives):
  - Host bins the 65536 nodes into 8*GP groups of <=128 nodes such that each
    group receives <=512 scatter contributions (edge endpoints). All
    gather/scatter index structure is folded into host-prepared transposed DMA
    slabs; the device does dense fp32r matmuls, a banded mask-matmul segment
    reduction, and GRU pointwise math.
  - The message second layer is folded into the GRU input weights on the host
    (Wfused = Wih @ ml2W), so the device scatters tanh activations and the
    ml2 bias becomes a rank-1 deg x (Wih @ ml2_b) term that rides a K=1
    matmul. All other biases also ride K=1 matmuls straight into PSUM.
  - Launch A runs propagation step 0; host reassembles node state and
    regathers; launch B runs step 1 plus the two gated aggregators pooled
    per-graph with owner-mask matmuls (accumulated in PSUM across all
    groups). Tiny heads finish on host.
"""

import os
import numpy as np
from contextlib import ExitStack

import concourse.tile as tile
from concourse import bacc, mybir
from concourse.bass_utils import run_bass_kernel_spmd
from concourse.masks import make_identity

F32 = mybir.dt.float32
F32R = mybir.dt.float32r
AF = mybir.ActivationFunctionType

B, N, E, S, A, T = 512, 65536, 131072, 256, 512, 64
NCORES = 8
CAP = 512          # contribution slots per group (4 subchunks of 128)
NSUB = CAP // 128

# rows2 layout (K=2 bias matmuls against [deg; ones]): [2, 3072]
R_X = 0            # 512: row0 bvec rz, row1 bhh' rz
R_Y = 512          # 256: row0 bvec n, row1 bih n
R_Z = 768          # 256: row0 bhh n (K=1 with ones)
R_AGG = 1024       # 4*512 on row0: dec_t_b, dec_g_b, init_t_b, init_g_b
R_LEN = 3072

_progs = {}
last_exec_ns = None


# ----------------------------------------------------------------------------
# Device program
# ----------------------------------------------------------------------------

def _build_prog(GP, GPC, with_agg):
    nc = bacc.Bacc("TRN2", target_bir_lowering=False, debug=False,
                   num_devices=NCORES)

    xcat = nc.dram_tensor("xcat", [128, GP, 4, CAP], F32R, kind="ExternalInput").ap()
    tpos = nc.dram_tensor("tpos", [128, GP * NSUB], F32, kind="ExternalInput").ap()
    ndT = nc.dram_tensor("ndT", [128, GP, 2, 128], F32R, kind="ExternalInput").ap()
    ndU = nc.dram_tensor("ndU", [128, GP, 256], F32, kind="ExternalInput").ap()
    upd = nc.dram_tensor("upd", [128, GP], F32, kind="ExternalInput").ap()
    degr = nc.dram_tensor("degr", [2, GP * 128], F32R, kind="ExternalInput").ap()
    rows = nc.dram_tensor("rows", [2, R_LEN], F32R, kind="ExternalInput").ap()
    wcat = nc.dram_tensor("wcat", [128, 4, 512], F32R, kind="ExternalInput").ap()
    wfu = nc.dram_tensor("wfu", [128, 4, 768], F32R, kind="ExternalInput").ap()
    whh = nc.dram_tensor("whh", [128, 2, 768], F32R, kind="ExternalInput").ap()
    if with_agg:
        omsk = nc.dram_tensor("omsk", [128, GP, 128], F32R,
                              kind="ExternalInput").ap()
        wagg = nc.dram_tensor("wagg", [128, 2, 4, 512], F32R,
                              kind="ExternalInput").ap()
        pool_out = nc.dram_tensor("pool_out", [128, 8, 512], F32,
                                  kind="ExternalOutput").ap()
        n2Td = nc.dram_tensor("n2Td", [128, GP, 2, 128], F32R).ap()
    else:
        newnodes = nc.dram_tensor("newnodes", [GP * 128, 256], F32,
                                  kind="ExternalOutput").ap()

    with tile.TileContext(nc) as tc, ExitStack() as ctx:
        consts = ctx.enter_context(tc.tile_pool(name="consts", bufs=1))
        ident = consts.tile([128, 128], F32)
        make_identity(nc, ident[:])
        iota0 = consts.tile([128, 128], F32)
        nc.gpsimd.iota(iota0[:], pattern=[[1, 128]], base=0, channel_multiplier=0,
                       allow_small_or_imprecise_dtypes=True)
        rows_t = consts.tile([2, R_LEN], F32R)
        nc.sync.dma_start(rows_t[:], rows[:])
        degr_t = consts.tile([2, GP * 128], F32R)
        nc.sync.dma_start(degr_t[:], degr[:])
        wcat_t = consts.tile([128, 4, 512], F32R)
        nc.sync.dma_start(wcat_t[:], wcat[:])
        wfu_t = consts.tile([128, 4, 768], F32R)
        nc.sync.dma_start(wfu_t[:], wfu[:])
        whh_t = consts.tile([128, 2, 768], F32R)
        nc.sync.dma_start(whh_t[:], whh[:])
        tpos_t = consts.tile([128, GP * NSUB], F32)
        nc.sync.dma_start(tpos_t[:], tpos[:])
        upd_t = consts.tile([128, GP], F32)
        nc.sync.dma_start(upd_t[:], upd[:])

        with ExitStack() as p1:
            slab_p = p1.enter_context(tc.tile_pool(name="slab", bufs=3))
            tpre_p = p1.enter_context(tc.tile_pool(name="tpre", bufs=2))
            mask_p = p1.enter_context(tc.tile_pool(name="mask", bufs=3))
            nd_p = p1.enter_context(tc.tile_pool(name="nd", bufs=3))
            gru_p = p1.enter_context(tc.tile_pool(name="gru", bufs=3))
            inpT_p = p1.enter_context(tc.tile_pool(name="inpT", bufs=2))
            out_p = p1.enter_context(tc.tile_pool(name="outp", bufs=3))
            pre_ps = p1.enter_context(tc.tile_pool(name="pre_ps", bufs=2,
                                                   space="PSUM"))
            scat_ps = p1.enter_context(tc.tile_pool(name="scat_ps", bufs=2,
                                                    space="PSUM"))
            x_ps = p1.enter_context(tc.tile_pool(name="x_ps", bufs=2,
                                                 space="PSUM"))
            yz_ps = p1.enter_context(tc.tile_pool(name="yz_ps", bufs=1,
                                                  space="PSUM"))

            for g in range(GP):
                slab = slab_p.tile([128, 4, CAP], F32R)
                nc.sync.dma_start(slab[:], xcat[:, g, :, :])

                # tanh(pre) per 128-contribution subchunk, token-major
                tpre = tpre_p.tile([128, 4, 512], F32R)
                scat = scat_ps.tile([128, 512], F32)
                for s in range(NSUB):
                    pp = pre_ps.tile([128, 512], F32)
                    for t in range(4):
                        nc.tensor.matmul(
                            pp[:], lhsT=slab[:, t, s * 128:(s + 1) * 128],
                            rhs=wcat_t[:, t, :], start=(t == 0), stop=(t == 3))
                    nc.scalar.activation(tpre[:, s, :], pp[:], AF.Tanh)
                    mask = mask_p.tile([128, 128], F32R)
                    nc.vector.tensor_tensor(
                        out=mask[:],
                        in0=tpos_t[:, g * NSUB + s:g * NSUB + s + 1]
                        .to_broadcast([128, 128]),
                        in1=iota0[:], op=mybir.AluOpType.is_equal)
                    nc.tensor.matmul(scat[:], lhsT=mask[:], rhs=tpre[:, s, :],
                                     start=(s == 0), stop=(s == NSUB - 1))

                # pooled tanh [node, 512] -> transposed inpT[p, t, m]
                inp_u = out_p.tile([128, 512], F32)
                nc.scalar.activation(inp_u[:], scat[:], AF.Copy)
                tp = x_ps.tile([128, 512], F32, tag="xps")
                for t in range(4):
                    nc.tensor.transpose(tp[:, t * 128:(t + 1) * 128],
                                        inp_u[:, t * 128:(t + 1) * 128], ident[:])
                inpT = inpT_p.tile([128, 4, 128], F32R)
                nc.vector.tensor_copy(
                    inpT[:], tp[:].rearrange("p (t n) -> p t n", t=4))

                ndT_g = nd_p.tile([128, 2, 128], F32R)
                nc.sync.dma_start(ndT_g[:], ndT[:, g, :, :])
                ndU_g = nd_p.tile([128, 256], F32)
                nc.sync.dma_start(ndU_g[:], ndU[:, g, :])

                dg = degr_t[:, g * 128:(g + 1) * 128]
                # X = (ir+hr | iz+hz), Y = inn, Z = hn -- biases via K=2 mm
                X = x_ps.tile([128, 512], F32, tag="xps")
                nc.tensor.matmul(X[:], lhsT=dg, rhs=rows_t[:, R_X:R_X + 512],
                                 start=True, stop=False)
                for t in range(4):
                    nc.tensor.matmul(X[:], lhsT=inpT[:, t, :],
                                     rhs=wfu_t[:, t, 0:512],
                                     start=False, stop=False)
                for t in range(2):
                    nc.tensor.matmul(X[:], lhsT=ndT_g[:, t, :],
                                     rhs=whh_t[:, t, 0:512],
                                     start=False, stop=(t == 1))
                Y = yz_ps.tile([128, 256], F32, tag="y")
                nc.tensor.matmul(Y[:], lhsT=dg, rhs=rows_t[:, R_Y:R_Y + 256],
                                 start=True, stop=False)
                for t in range(4):
                    nc.tensor.matmul(Y[:], lhsT=inpT[:, t, :],
                                     rhs=wfu_t[:, t, 512:768],
                                     start=False, stop=(t == 3))
                Z = yz_ps.tile([128, 256], F32, tag="z")
                nc.tensor.matmul(Z[:], lhsT=dg, rhs=rows_t[:, R_Z:R_Z + 256],
                                 start=True, stop=False)
                for t in range(2):
                    nc.tensor.matmul(Z[:], lhsT=ndT_g[:, t, :],
                                     rhs=whh_t[:, t, 512:768],
                                     start=False, stop=(t == 1))

                r = gru_p.tile([128, 256], F32)
                nc.scalar.activation(r[:], X[:, 0:256], AF.Sigmoid)
                zp = gru_p.tile([128, 256], F32)   # 1 - z = sigmoid(-(iz+hz))
                nc.scalar.activation(zp[:], X[:, 256:512], AF.Sigmoid, scale=-1.0)
                tn = gru_p.tile([128, 256], F32)
                nc.vector.tensor_mul(tn[:], r[:], Z[:])
                nc.vector.tensor_add(tn[:], tn[:], Y[:])
                nn = gru_p.tile([128, 256], F32)
                nc.scalar.activation(nn[:], tn[:], AF.Tanh)
                um = gru_p.tile([128, 256], F32)
                nc.vector.tensor_tensor(
                    out=um[:], in0=zp[:],
                    in1=upd_t[:, g:g + 1].to_broadcast([128, 256]),
                    op=mybir.AluOpType.mult)
                dnh = gru_p.tile([128, 256], F32)
                nc.vector.tensor_sub(dnh[:], nn[:], ndU_g[:])
                nc.vector.tensor_mul(dnh[:], dnh[:], um[:])
                newn = out_p.tile([128, 256], F32)
                nc.vector.tensor_add(newn[:], ndU_g[:], dnh[:])

                if with_agg:
                    n2ps = yz_ps.tile([128, 256], F32, tag="y")
                    for t in range(2):
                        nc.tensor.transpose(n2ps[:, t * 128:(t + 1) * 128],
                                            newn[:, t * 128:(t + 1) * 128],
                                            ident[:])
                    n2T = inpT_p.tile([128, 2, 128], F32R)
                    nc.vector.tensor_copy(
                        n2T[:], n2ps[:].rearrange("p (t n) -> p t n", t=2))
                    nc.sync.dma_start(n2Td[:, g, :, :], n2T[:])
                else:
                    nc.sync.dma_start(newnodes[g * 128:(g + 1) * 128, :], newn[:])

        if with_agg:
            with ExitStack() as p2:
                agg_c = p2.enter_context(tc.tile_pool(name="agg_c", bufs=1))
                wagg_t = agg_c.tile([128, 2, 4, 512], F32R)
                nc.sync.dma_start(wagg_t[:], wagg[:])
                pooled_sb = agg_c.tile([128, 8, 512], F32)

                a_sb = p2.enter_context(tc.tile_pool(name="a_sb", bufs=4))
                a_m = p2.enter_context(tc.tile_pool(name="a_m", bufs=3))
                a_ps = p2.enter_context(tc.tile_pool(name="a_ps", bufs=6,
                                                     space="PSUM"))
                p_ps = p2.enter_context(tc.tile_pool(name="p_ps", bufs=2,
                                                     space="PSUM"))
                for a in range(2):
                    pq = None
                    for g in range(GP):
                        if g % GPC == 0:
                            pq = p_ps.tile([128, 512], F32, tag="pq",
                                           name=f"pq{a}_{g // GPC}")
                        n2g = a_sb.tile([128, 2, 128], F32R)
                        nc.sync.dma_start(n2g[:], n2Td[:, g, :, :])
                        omask = a_m.tile([128, 128], F32R)
                        nc.sync.dma_start(omask[:], omsk[:, g, :])
                        dp = a_ps.tile([128, 512], F32, tag="aps")
                        for t in range(2):
                            nc.tensor.matmul(dp[:], lhsT=n2g[:, t, :],
                                             rhs=wagg_t[:, t, 2 * a, :],
                                             start=(t == 0), stop=False)
                        nc.tensor.matmul(
                            dp[:], lhsT=degr_t[:, 0:128],
                            rhs=rows_t[:, R_AGG + (2 * a) * 512:
                                       R_AGG + (2 * a + 1) * 512],
                            start=False, stop=True)
                        gp2 = a_ps.tile([128, 512], F32, tag="aps")
                        for t in range(2):
                            nc.tensor.matmul(gp2[:], lhsT=n2g[:, t, :],
                                             rhs=wagg_t[:, t, 2 * a + 1, :],
                                             start=(t == 0), stop=False)
                        nc.tensor.matmul(
                            gp2[:], lhsT=degr_t[:, 0:128],
                            rhs=rows_t[:, R_AGG + (2 * a + 1) * 512:
                                       R_AGG + (2 * a + 2) * 512],
                            start=False, stop=True)
                        gates = a_sb.tile([128, 512], F32)
                        nc.scalar.activation(gates[:], gp2[:], AF.Sigmoid)
                        gated = a_sb.tile([128, 512], F32R)
                        nc.vector.tensor_mul(gated[:], dp[:], gates[:])
                        nc.tensor.matmul(pq[:], lhsT=omask[:],
                                         rhs=gated[:], start=(g % GPC == 0),
                                         stop=(g % GPC == GPC - 1))
                        if g % GPC == GPC - 1:
                            nc.vector.tensor_copy(
                                pooled_sb[:, 4 * a + g // GPC, :], pq[:])
                nc.sync.dma_start(pool_out[:], pooled_sb[:])

    nc.compile()
    return nc


# ----------------------------------------------------------------------------
# Host-side index structure and slab packing
# ----------------------------------------------------------------------------

def _wT3(W, kt):
    # W [fout, fin] -> [128, kt, fout] with [p, t, f] = W[f, 128*t + p]
    fout = W.shape[0]
    return np.ascontiguousarray(
        W.T.reshape(kt, 128, fout).transpose(1, 0, 2)).astype(np.float32)


def _structure(edge_source, edge_dest, node_owner, running):
    deg = (np.bincount(edge_source, minlength=N)
           + np.bincount(edge_dest, minlength=N)).astype(np.int64)
    assert deg.max() <= CAP
    # bins are constrained to a single owner-class (owner // 128) so each
    # group pools into exactly one owner-tile in phase 2
    cls = (node_owner // 128).astype(np.int64)
    degl = deg.tolist()
    bins_per_class = [[] for _ in range(4)]
    node_bin_seq = np.empty(N, np.int32)   # (class-local bin index)
    pos = np.empty(N, np.int32)
    for k in range(4):
        nodes_k = np.nonzero(cls == k)[0]
        bl = bins_per_class[k]
        cnt = 128
        csum = 0
        bidx = -1
        for n in nodes_k.tolist():
            d = degl[n]
            if cnt >= 128 or csum + d > CAP:
                bidx += 1
                bl.append(bidx)
                cnt = 0
                csum = 0
            node_bin_seq[n] = bidx
            pos[n] = cnt
            cnt += 1
            csum += d
    nb_k = [len(bins_per_class[k]) for k in range(4)]
    GPC = max(-(-nk // NCORES) for nk in nb_k)
    GP = 4 * GPC
    # class-k bin j -> core j % 8, group k*GPC + j//8 ; global bin id
    bin_id = np.empty(N, np.int32)
    for k in range(4):
        sel = cls == k
        j = node_bin_seq[sel]
        core = j % NCORES
        grp = k * GPC + j // NCORES
        bin_id[sel] = core * GP + grp
    nbins = NCORES * GP

    tgt = np.concatenate([edge_dest, edge_source])
    eid = np.concatenate([np.arange(E, dtype=np.int64)] * 2)
    tb = bin_id[tgt]
    order = np.argsort(tb, kind="stable")
    tb_s = tb[order]
    eid_s = eid[order]
    tpos_s = pos[tgt][order].astype(np.float32)
    counts = np.bincount(tb_s, minlength=NCORES * GP)
    assert counts.max() <= CAP
    starts = np.concatenate([[0], np.cumsum(counts)])

    L = GP * CAP
    e_slot = np.zeros((NCORES, L), np.int64)
    tp_slot = np.full((NCORES, L), 300.0, np.float32)
    for bb in range(nbins):
        c, g = divmod(bb, GP)
        s0 = int(starts[bb])
        n = int(counts[bb])
        e_slot[c, g * CAP:g * CAP + n] = eid_s[s0:s0 + n]
        tp_slot[c, g * CAP:g * CAP + n] = tpos_s[s0:s0 + n]

    slot_node = np.full((NCORES, GP * 128), -1, np.int64)
    core_of = bin_id // GP
    slot_of = (bin_id % GP) * 128 + pos
    slot_node[core_of, slot_of] = np.arange(N)

    run_f = np.asarray(running, bool)
    tpos_slab, upd_slab, own_slab, deg_slab = [], [], [], []
    for c in range(NCORES):
        tpos_slab.append(np.ascontiguousarray(
            tp_slot[c].reshape(GP, NSUB, 128).transpose(2, 0, 1)
            .reshape(128, GP * NSUB)))
        sn = slot_node[c]
        real = sn >= 0
        snc = np.maximum(sn, 0)
        u = (run_f[node_owner[snc]] & real).astype(np.float32)
        upd_slab.append(np.ascontiguousarray(u.reshape(GP, 128).T))
        ow = np.where(real, node_owner[snc], 10000).reshape(GP, 128)
        gcls = (np.arange(GP) // (GP // 4))[:, None]
        om = (ow[:, :, None]
              == (gcls[:, :, None] * 128 + np.arange(128)[None, None, :]))
        own_slab.append(np.ascontiguousarray(
            om.transpose(1, 0, 2).astype(np.float32)))
        dg = np.where(real, deg[snc], 0).astype(np.float32)
        deg_slab.append(np.ascontiguousarray(
            np.stack([dg, np.ones_like(dg)], axis=0)))
    return dict(GP=GP, GPC=GPC, e_slot=e_slot, slot_node=slot_node,
                tpos_slab=tpos_slab, upd_slab=upd_slab, own_slab=own_slab,
                deg_slab=deg_slab)


def _step_slabs(st, nodes_cur, edge_features, edge_source, edge_dest):
    GP = st["GP"]
    L = GP * CAP
    xcats, ndTs, ndUs = [], [], []
    for c in range(NCORES):
        es = st["e_slot"][c]
        ns = nodes_cur[edge_source[es]] + nodes_cur[edge_dest[es]]
        ef = edge_features[es]
        x1 = ns.reshape(L, 2, 128).transpose(2, 1, 0)
        x2 = ef.reshape(L, 2, 128).transpose(2, 1, 0)
        xc = np.concatenate([x1, x2], axis=1)          # [128, 4, L]
        xc = xc.reshape(128, 4, GP, CAP).transpose(0, 2, 1, 3)
        xcats.append(np.ascontiguousarray(xc))

        sn = st["slot_node"][c]
        nd = nodes_cur[np.maximum(sn, 0)] * (sn >= 0)[:, None]
        ndT = nd.reshape(GP, 128, 2, 128).transpose(3, 0, 2, 1)
        ndTs.append(np.ascontiguousarray(ndT))
        ndU = nd.reshape(GP, 128, 256).transpose(1, 0, 2)
        ndUs.append(np.ascontiguousarray(ndU))
    return xcats, ndTs, ndUs


def _step_weights(inp, p):
    wc = np.concatenate([inp["mn_W"][p], inp["mf_W"][p]], axis=1)  # [512, 512]
    ml2 = inp["ml2_W"][p].astype(np.float32)
    wih = inp["gru_Wih"][p].astype(np.float32)
    wfused = wih @ ml2                                  # [768, 512]
    bvec = wih @ inp["ml2_b"][p].astype(np.float32)     # [768]
    bih = inp["gru_bih"][p].astype(np.float32)
    bhh = inp["gru_bhh"][p].astype(np.float32)
    bhh2 = bhh.copy()
    bhh2[0:512] += bih[0:512]                           # fold r,z input bias
    rows = np.zeros((2, R_LEN), np.float32)
    rows[0, R_X:R_X + 512] = bvec[0:512]
    rows[1, R_X:R_X + 512] = bhh2[0:512]
    rows[0, R_Y:R_Y + 256] = bvec[512:768]
    rows[1, R_Y:R_Y + 256] = bih[512:768]
    rows[1, R_Z:R_Z + 256] = bhh2[512:768]
    rows[1, R_AGG + 0 * 512:R_AGG + 1 * 512] = inp["dec_t_b"]
    rows[1, R_AGG + 1 * 512:R_AGG + 2 * 512] = inp["dec_g_b"]
    rows[1, R_AGG + 2 * 512:R_AGG + 3 * 512] = inp["init_t_b"]
    rows[1, R_AGG + 3 * 512:R_AGG + 4 * 512] = inp["init_g_b"]
    return dict(
        wcat=_wT3(wc, 4),
        wfu=_wT3(wfused, 4),
        whh=_wT3(inp["gru_Whh"][p].astype(np.float32), 2),
        rows=rows,
    )


# ----------------------------------------------------------------------------
# Entry point
# ----------------------------------------------------------------------------

def kernel(**inputs):
    global last_exec_ns
    inp = {k: np.asarray(v) for k, v in inputs.items()}
    nodes0 = inp["nodes"].astype(np.float32)
    ef = inp["edge_features"].astype(np.float32)
    esrc = inp["edge_source"].astype(np.int64)
    edst = inp["edge_dest"].astype(np.int64)

    st = _structure(esrc, edst, inp["node_owner"].astype(np.int64),
                    inp["running"])
    GP = st["GP"]
    GPC = st["GPC"]

    trace = bool(os.environ.get("BASSK_TRACE"))
    if trace:
        _install_trace_hook()
    exec_ns = []

    def run(prog, maps):
        res = run_bass_kernel_spmd(prog, maps, list(range(NCORES)), trace=trace)
        exec_ns.append(res.exec_time_ns)
        return res.results

    # ---- launch A: step 0 ----
    if (GP, GPC, False) not in _progs:
        _progs[(GP, GPC, False)] = _build_prog(GP, GPC, False)
    w0 = _step_weights(inp, 0)
    xcats, ndTs, ndUs = _step_slabs(st, nodes0, ef, esrc, edst)
    maps = [dict(xcat=xcats[c], tpos=st["tpos_slab"][c], ndT=ndTs[c],
                 ndU=ndUs[c], upd=st["upd_slab"][c], degr=st["deg_slab"][c],
                 **w0)
            for c in range(NCORES)]
    resA = run(_progs[(GP, GPC, False)], maps)

    nodes1 = np.array(nodes0)
    for c in range(NCORES):
        sn = st["slot_node"][c]
        real = sn >= 0
        nodes1[sn[real]] = resA[c]["newnodes"][real]

    # ---- launch B: step 1 + aggregators ----
    if (GP, GPC, True) not in _progs:
        _progs[(GP, GPC, True)] = _build_prog(GP, GPC, True)
    w1 = _step_weights(inp, 1)
    wagg = np.ascontiguousarray(np.stack(
        [_wT3(inp["dec_t_W"].astype(np.float32), 2),
         _wT3(inp["dec_g_W"].astype(np.float32), 2),
         _wT3(inp["init_t_W"].astype(np.float32), 2),
         _wT3(inp["init_g_W"].astype(np.float32), 2)], axis=2))
    xcats, ndTs, ndUs = _step_slabs(st, nodes1, ef, esrc, edst)
    maps = [dict(xcat=xcats[c], tpos=st["tpos_slab"][c], ndT=ndTs[c],
                 ndU=ndUs[c], upd=st["upd_slab"][c], degr=st["deg_slab"][c],
                 omsk=st["own_slab"][c], wagg=wagg, **w1)
            for c in range(NCORES)]
    resB = run(_progs[(GP, GPC, True)], maps)

    pool_sum = np.zeros((128, 8, 512), np.float32)
    for c in range(NCORES):
        pool_sum += resB[c]["pool_out"]
    agg_dec = pool_sum[:, 0:4, :].transpose(1, 0, 2).reshape(B, A)
    agg_init = pool_sum[:, 4:8, :].transpose(1, 0, 2).reshape(B, A)

    # ---- tiny heads on host ----
    logits = agg_dec @ inp["ntd_W"].astype(np.float32).T \
        + inp["ntd_b"].astype(np.float32)
    sel = inp["ref_types"].astype(np.int64) + 1
    m = logits.max(axis=-1, keepdims=True)
    lse = m + np.log(np.exp(logits - m).sum(axis=-1, keepdims=True))
    logp = logits - lse
    per_ex = -logp[np.arange(B), sel]
    loss = np.where(np.asarray(inp["running"], bool), per_ex,
                    0.0).astype(np.float32).mean()
    emb = inp["nte"].astype(np.float32)[sel - 1]
    new_features = emb @ inp["f1_W"].astype(np.float32).T \
        + inp["f1_b"].astype(np.float32) \
        + agg_init @ inp["f2_W"].astype(np.float32).T

    if trace:
        last_exec_ns = exec_ns
    return (logits.astype(np.float32), new_features.astype(np.float32),
            np.float32(loss))


def _install_trace_hook():
    import sys
    import types
    if "antenv.axon_hooks" in sys.modules:
        return
    try:
        from trn_agent_boot.trn_boot import _ntff_profile_via_ctypes
        hook = _ntff_profile_via_ctypes("/opt/axon/libaxon_pjrt.so")
    except Exception:
        hook = None
    mod = types.ModuleType("antenv.axon_hooks")
    mod.get_axon_ntff_profile_hook = lambda: hook
    mod.set_axon_ntff_profile_hook = lambda h: None
    sys.modules["antenv.axon_hooks"] = mod


# revision 11
# speedup vs baseline: 1.0001x; 1.0001x over previous
"""Trainium2 Bass kernel for nn_NodeAdder (GGNN propagation + node-type head).

Strategy (8 NeuronCores, SPMD, no collectives):
  - Host bins the 65536 nodes into 8*GP groups of <=128 nodes such that each
    group receives <=512 scatter contributions (edge endpoints). All
    gather/scatter index structure is folded into host-prepared transposed DMA
    slabs; the device does dense fp32r matmuls, a banded mask-matmul segment
    reduction, and GRU pointwise math.
  - The message second layer is folded into the GRU input weights on the host
    (Wfused = Wih @ ml2W), so the device scatters tanh activations and the
    ml2 bias becomes a rank-1 deg x (Wih @ ml2_b) term that rides a K=1
    matmul. All other biases also ride K=1 matmuls straight into PSUM.
  - Launch A runs propagation step 0; host reassembles node state and
    regathers; launch B runs step 1 plus the two gated aggregators pooled
    per-graph with owner-mask matmuls (accumulated in PSUM across all
    groups). Tiny heads finish on host.
"""

import os
import numpy as np
from contextlib import ExitStack

import concourse.tile as tile
from concourse import bacc, mybir
from concourse.bass_utils import run_bass_kernel_spmd
from concourse.masks import make_identity

F32 = mybir.dt.float32
F32R = mybir.dt.float32r
AF = mybir.ActivationFunctionType

B, N, E, S, A, T = 512, 65536, 131072, 256, 512, 64
NCORES = 8
CAP = 512          # contribution slots per group (4 subchunks of 128)
NSUB = CAP // 128

# rows2 layout (K=2 bias matmuls against [deg; ones]): [2, 3072]
R_X = 0            # 512: row0 bvec rz, row1 bhh' rz
R_Y = 512          # 256: row0 bvec n, row1 bih n
R_Z = 768          # 256: row0 bhh n (K=1 with ones)
R_AGG = 1024       # 4*512 on row0: dec_t_b, dec_g_b, init_t_b, init_g_b
R_LEN = 3072

_progs = {}
last_exec_ns = None


# ----------------------------------------------------------------------------
# Device program
# ----------------------------------------------------------------------------

def _build_prog(GP, GPC, with_agg):
    nc = bacc.Bacc("TRN2", target_bir_lowering=False, debug=False,
                   num_devices=NCORES)

    xcat = nc.dram_tensor("xcat", [128, GP, 4, CAP], F32R, kind="ExternalInput").ap()
    tpos = nc.dram_tensor("tpos", [128, GP * NSUB], F32, kind="ExternalInput").ap()
    ndT = nc.dram_tensor("ndT", [128, GP, 2, 128], F32R, kind="ExternalInput").ap()
    ndU = nc.dram_tensor("ndU", [128, GP, 256], F32, kind="ExternalInput").ap()
    upd = nc.dram_tensor("upd", [128, GP], F32, kind="ExternalInput").ap()
    degr = nc.dram_tensor("degr", [2, GP * 128], F32R, kind="ExternalInput").ap()
    rows = nc.dram_tensor("rows", [2, R_LEN], F32R, kind="ExternalInput").ap()
    wcat = nc.dram_tensor("wcat", [128, 4, 512], F32R, kind="ExternalInput").ap()
    wfu = nc.dram_tensor("wfu", [128, 4, 768], F32R, kind="ExternalInput").ap()
    whh = nc.dram_tensor("whh", [128, 2, 768], F32R, kind="ExternalInput").ap()
    if with_agg:
        omsk = nc.dram_tensor("omsk", [128, GP, 128], F32R,
                              kind="ExternalInput").ap()
        wagg = nc.dram_tensor("wagg", [128, 2, 4, 512], F32R,
                              kind="ExternalInput").ap()
        pool_out = nc.dram_tensor("pool_out", [128, 8, 512], F32,
                                  kind="ExternalOutput").ap()
        n2Td = nc.dram_tensor("n2Td", [128, GP, 2, 128], F32R).ap()
    else:
        newnodes = nc.dram_tensor("newnodes", [GP * 128, 256], F32,
                                  kind="ExternalOutput").ap()

    with tile.TileContext(nc) as tc, ExitStack() as ctx:
        consts = ctx.enter_context(tc.tile_pool(name="consts", bufs=1))
        ident = consts.tile([128, 128], F32)
        make_identity(nc, ident[:])
        iota0 = consts.tile([128, 128], F32)
        nc.gpsimd.iota(iota0[:], pattern=[[1, 128]], base=0, channel_multiplier=0,
                       allow_small_or_imprecise_dtypes=True)
        rows_t = consts.tile([2, R_LEN], F32R)
        nc.sync.dma_start(rows_t[:], rows[:])
        degr_t = consts.tile([2, GP * 128], F32R)
        nc.sync.dma_start(degr_t[:], degr[:])
        wcat_t = consts.tile([128, 4, 512], F32R)
        nc.sync.dma_start(wcat_t[:], wcat[:])
        wfu_t = consts.tile([128, 4, 768], F32R)
        nc.sync.dma_start(wfu_t[:], wfu[:])
        whh_t = consts.tile([128, 2, 768], F32R)
        nc.sync.dma_start(whh_t[:], whh[:])
        tpos_t = consts.tile([128, GP * NSUB], F32)
        nc.sync.dma_start(tpos_t[:], tpos[:])
        upd_t = consts.tile([128, GP], F32)
        nc.sync.dma_start(upd_t[:], upd[:])

        with ExitStack() as p1:
            slab_p = p1.enter_context(tc.tile_pool(name="slab", bufs=3))
            tpre_p = p1.enter_context(tc.tile_pool(name="tpre", bufs=2))
            mask_p = p1.enter_context(tc.tile_pool(name="mask", bufs=3))
            nd_p = p1.enter_context(tc.tile_pool(name="nd", bufs=3))
            gru_p = p1.enter_context(tc.tile_pool(name="gru", bufs=3))
            inpT_p = p1.enter_context(tc.tile_pool(name="inpT", bufs=2))
            out_p = p1.enter_context(tc.tile_pool(name="outp", bufs=3))
            pre_ps = p1.enter_context(tc.tile_pool(name="pre_ps", bufs=2,
                                                   space="PSUM"))
            scat_ps = p1.enter_context(tc.tile_pool(name="scat_ps", bufs=2,
                                                    space="PSUM"))
            x_ps = p1.enter_context(tc.tile_pool(name="x_ps", bufs=2,
                                                 space="PSUM"))
            yz_ps = p1.enter_context(tc.tile_pool(name="yz_ps", bufs=1,
                                                  space="PSUM"))

            for g in range(GP):
                slab = slab_p.tile([128, 4, CAP], F32R)
                nc.sync.dma_start(slab[:], xcat[:, g, :, :])

                # tanh(pre) per 128-contribution subchunk, token-major
                tpre = tpre_p.tile([128, 4, 512], F32R)
                scat = scat_ps.tile([128, 512], F32)
                for s in range(NSUB):
                    pp = pre_ps.tile([128, 512], F32)
                    for t in range(4):
                        nc.tensor.matmul(
                            pp[:], lhsT=slab[:, t, s * 128:(s + 1) * 128],
                            rhs=wcat_t[:, t, :], start=(t == 0), stop=(t == 3))
                    nc.scalar.activation(tpre[:, s, :], pp[:], AF.Tanh)
                    mask = mask_p.tile([128, 128], F32R)
                    nc.vector.tensor_tensor(
                        out=mask[:],
                        in0=tpos_t[:, g * NSUB + s:g * NSUB + s + 1]
                        .to_broadcast([128, 128]),
                        in1=iota0[:], op=mybir.AluOpType.is_equal)
                    nc.tensor.matmul(scat[:], lhsT=mask[:], rhs=tpre[:, s, :],
                                     start=(s == 0), stop=(s == NSUB - 1))

                # pooled tanh [node, 512] -> transposed inpT[p, t, m]
                inp_u = out_p.tile([128, 512], F32)
                nc.scalar.activation(inp_u[:], scat[:], AF.Copy)
                tp = x_ps.tile([128, 512], F32, tag="xps")
                for t in range(4):
                    nc.tensor.transpose(tp[:, t * 128:(t + 1) * 128],
                                        inp_u[:, t * 128:(t + 1) * 128], ident[:])
                inpT = inpT_p.tile([128, 4, 128], F32R)
                nc.vector.tensor_copy(
                    inpT[:], tp[:].rearrange("p (t n) -> p t n", t=4))

                ndT_g = nd_p.tile([128, 2, 128], F32R)
                nc.sync.dma_start(ndT_g[:], ndT[:, g, :, :])
                ndU_g = nd_p.tile([128, 256], F32)
                nc.sync.dma_start(ndU_g[:], ndU[:, g, :])

                dg = degr_t[:, g * 128:(g + 1) * 128]
                # X = (ir+hr | iz+hz), Y = inn, Z = hn -- biases via K=2 mm
                X = x_ps.tile([128, 512], F32, tag="xps")
                nc.tensor.matmul(X[:], lhsT=dg, rhs=rows_t[:, R_X:R_X + 512],
                                 start=True, stop=False)
                for t in range(4):
                    nc.tensor.matmul(X[:], lhsT=inpT[:, t, :],
                                     rhs=wfu_t[:, t, 0:512],
                                     start=False, stop=False)
                for t in range(2):
                    nc.tensor.matmul(X[:], lhsT=ndT_g[:, t, :],
                                     rhs=whh_t[:, t, 0:512],
                                     start=False, stop=(t == 1))
                Y = yz_ps.tile([128, 256], F32, tag="y")
                nc.tensor.matmul(Y[:], lhsT=dg, rhs=rows_t[:, R_Y:R_Y + 256],
                                 start=True, stop=False)
                for t in range(4):
                    nc.tensor.matmul(Y[:], lhsT=inpT[:, t, :],
                                     rhs=wfu_t[:, t, 512:768],
                                     start=False, stop=(t == 3))
                Z = yz_ps.tile([128, 256], F32, tag="z")
                nc.tensor.matmul(Z[:], lhsT=dg, rhs=rows_t[:, R_Z:R_Z + 256],
                                 start=True, stop=False)
                for t in range(2):
                    nc.tensor.matmul(Z[:], lhsT=ndT_g[:, t, :],
                                     rhs=whh_t[:, t, 512:768],
                                     start=False, stop=(t == 1))

                r = gru_p.tile([128, 256], F32)
                nc.scalar.activation(r[:], X[:, 0:256], AF.Sigmoid)
                zp = gru_p.tile([128, 256], F32)   # 1 - z = sigmoid(-(iz+hz))
                nc.scalar.activation(zp[:], X[:, 256:512], AF.Sigmoid, scale=-1.0)
                tn = gru_p.tile([128, 256], F32)
                nc.vector.tensor_mul(tn[:], r[:], Z[:])
                nc.vector.tensor_add(tn[:], tn[:], Y[:])
                nn = gru_p.tile([128, 256], F32)
                nc.scalar.activation(nn[:], tn[:], AF.Tanh)
                um = gru_p.tile([128, 256], F32)
                nc.vector.tensor_tensor(
                    out=um[:], in0=zp[:],
                    in1=upd_t[:, g:g + 1].to_broadcast([128, 256]),
                    op=mybir.AluOpType.mult)
                dnh = gru_p.tile([128, 256], F32)
                nc.vector.tensor_sub(dnh[:], nn[:], ndU_g[:])
                nc.vector.tensor_mul(dnh[:], dnh[:], um[:])
                newn = out_p.tile([128, 256], F32)
                nc.vector.tensor_add(newn[:], ndU_g[:], dnh[:])

                if with_agg:
                    n2ps = yz_ps.tile([128, 256], F32, tag="y")
                    for t in range(2):
                        nc.tensor.transpose(n2ps[:, t * 128:(t + 1) * 128],
                                            newn[:, t * 128:(t + 1) * 128],
                                            ident[:])
                    n2T = inpT_p.tile([128, 2, 128], F32R)
                    nc.vector.tensor_copy(
                        n2T[:], n2ps[:].rearrange("p (t n) -> p t n", t=2))
                    nc.sync.dma_start(n2Td[:, g, :, :], n2T[:])
                else:
                    nc.sync.dma_start(newnodes[g * 128:(g + 1) * 128, :], newn[:])

        if with_agg:
            with ExitStack() as p2:
                agg_c = p2.enter_context(tc.tile_pool(name="agg_c", bufs=1))
                wagg_t = agg_c.tile([128, 2, 4, 512], F32R)
                nc.sync.dma_start(wagg_t[:], wagg[:])
                pooled_sb = agg_c.tile([128, 8, 512], F32)

                a_sb = p2.enter_context(tc.tile_pool(name="a_sb", bufs=4))
                a_m = p2.enter_context(tc.tile_pool(name="a_m", bufs=3))
                a_ps = p2.enter_context(tc.tile_pool(name="a_ps", bufs=4,
                                                     space="PSUM"))
                p_ps = p2.enter_context(tc.tile_pool(name="p_ps", bufs=2,
                                                     space="PSUM"))
                pqd = pqi = None
                for g in range(GP):
                    if g % GPC == 0:
                        pqd = p_ps.tile([128, 512], F32, tag="pqd",
                                        name=f"pqd{g // GPC}")
                        pqi = p_ps.tile([128, 512], F32, tag="pqi",
                                        name=f"pqi{g // GPC}")
                    n2g = a_sb.tile([128, 2, 128], F32R)
                    nc.sync.dma_start(n2g[:], n2Td[:, g, :, :])
                    omask = a_m.tile([128, 128], F32R)
                    nc.sync.dma_start(omask[:], omsk[:, g, :])
                    gated2 = []
                    for a in range(2):
                        dp = a_ps.tile([128, 512], F32, tag="aps",
                                       name=f"dp{a}")
                        for t in range(2):
                            nc.tensor.matmul(dp[:], lhsT=n2g[:, t, :],
                                             rhs=wagg_t[:, t, 2 * a, :],
                                             start=(t == 0), stop=False)
                        nc.tensor.matmul(
                            dp[:], lhsT=degr_t[:, 0:128],
                            rhs=rows_t[:, R_AGG + (2 * a) * 512:
                                       R_AGG + (2 * a + 1) * 512],
                            start=False, stop=True)
                        gp2 = a_ps.tile([128, 512], F32, tag="aps",
                                        name=f"gp{a}")
                        for t in range(2):
                            nc.tensor.matmul(gp2[:], lhsT=n2g[:, t, :],
                                             rhs=wagg_t[:, t, 2 * a + 1, :],
                                             start=(t == 0), stop=False)
                        nc.tensor.matmul(
                            gp2[:], lhsT=degr_t[:, 0:128],
                            rhs=rows_t[:, R_AGG + (2 * a + 1) * 512:
                                       R_AGG + (2 * a + 2) * 512],
                            start=False, stop=True)
                        gates = a_sb.tile([128, 512], F32, tag="gates",
                                          name=f"gates{a}")
                        nc.scalar.activation(gates[:], gp2[:], AF.Sigmoid)
                        gated = a_sb.tile([128, 512], F32R, tag="gated",
                                          name=f"gated{a}")
                        nc.vector.tensor_mul(gated[:], dp[:], gates[:])
                        gated2.append(gated)
                    for a, pq in ((0, pqd), (1, pqi)):
                        nc.tensor.matmul(pq[:], lhsT=omask[:],
                                         rhs=gated2[a][:],
                                         start=(g % GPC == 0),
                                         stop=(g % GPC == GPC - 1))
                    if g % GPC == GPC - 1:
                        nc.vector.tensor_copy(
                            pooled_sb[:, 0 + g // GPC, :], pqd[:])
                        nc.vector.tensor_copy(
                            pooled_sb[:, 4 + g // GPC, :], pqi[:])
                nc.sync.dma_start(pool_out[:], pooled_sb[:])

    nc.compile()
    return nc


# ----------------------------------------------------------------------------
# Host-side index structure and slab packing
# ----------------------------------------------------------------------------

def _wT3(W, kt):
    # W [fout, fin] -> [128, kt, fout] with [p, t, f] = W[f, 128*t + p]
    fout = W.shape[0]
    return np.ascontiguousarray(
        W.T.reshape(kt, 128, fout).transpose(1, 0, 2)).astype(np.float32)


def _structure(edge_source, edge_dest, node_owner, running):
    deg = (np.bincount(edge_source, minlength=N)
           + np.bincount(edge_dest, minlength=N)).astype(np.int64)
    assert deg.max() <= CAP
    # bins are constrained to a single owner-class (owner // 128) so each
    # group pools into exactly one owner-tile in phase 2
    cls = (node_owner // 128).astype(np.int64)
    degl = deg.tolist()
    bins_per_class = [[] for _ in range(4)]
    node_bin_seq = np.empty(N, np.int32)   # (class-local bin index)
    pos = np.empty(N, np.int32)
    for k in range(4):
        nodes_k = np.nonzero(cls == k)[0]
        bl = bins_per_class[k]
        cnt = 128
        csum = 0
        bidx = -1
        for n in nodes_k.tolist():
            d = degl[n]
            if cnt >= 128 or csum + d > CAP:
                bidx += 1
                bl.append(bidx)
                cnt = 0
                csum = 0
            node_bin_seq[n] = bidx
            pos[n] = cnt
            cnt += 1
            csum += d
    nb_k = [len(bins_per_class[k]) for k in range(4)]
    GPC = max(-(-nk // NCORES) for nk in nb_k)
    GP = 4 * GPC
    # class-k bin j -> core j % 8, group k*GPC + j//8 ; global bin id
    bin_id = np.empty(N, np.int32)
    for k in range(4):
        sel = cls == k
        j = node_bin_seq[sel]
        core = j % NCORES
        grp = k * GPC + j // NCORES
        bin_id[sel] = core * GP + grp
    nbins = NCORES * GP

    tgt = np.concatenate([edge_dest, edge_source])
    eid = np.concatenate([np.arange(E, dtype=np.int64)] * 2)
    tb = bin_id[tgt]
    order = np.argsort(tb, kind="stable")
    tb_s = tb[order]
    eid_s = eid[order]
    tpos_s = pos[tgt][order].astype(np.float32)
    counts = np.bincount(tb_s, minlength=NCORES * GP)
    assert counts.max() <= CAP
    starts = np.concatenate([[0], np.cumsum(counts)])

    L = GP * CAP
    e_slot = np.zeros((NCORES, L), np.int64)
    tp_slot = np.full((NCORES, L), 300.0, np.float32)
    for bb in range(nbins):
        c, g = divmod(bb, GP)
        s0 = int(starts[bb])
        n = int(counts[bb])
        e_slot[c, g * CAP:g * CAP + n] = eid_s[s0:s0 + n]
        tp_slot[c, g * CAP:g * CAP + n] = tpos_s[s0:s0 + n]

    slot_node = np.full((NCORES, GP * 128), -1, np.int64)
    core_of = bin_id // GP
    slot_of = (bin_id % GP) * 128 + pos
    slot_node[core_of, slot_of] = np.arange(N)

    run_f = np.asarray(running, bool)
    tpos_slab, upd_slab, own_slab, deg_slab = [], [], [], []
    for c in range(NCORES):
        tpos_slab.append(np.ascontiguousarray(
            tp_slot[c].reshape(GP, NSUB, 128).transpose(2, 0, 1)
            .reshape(128, GP * NSUB)))
        sn = slot_node[c]
        real = sn >= 0
        snc = np.maximum(sn, 0)
        u = (run_f[node_owner[snc]] & real).astype(np.float32)
        upd_slab.append(np.ascontiguousarray(u.reshape(GP, 128).T))
        ow = np.where(real, node_owner[snc], 10000).reshape(GP, 128)
        gcls = (np.arange(GP) // (GP // 4))[:, None]
        om = (ow[:, :, None]
              == (gcls[:, :, None] * 128 + np.arange(128)[None, None, :]))
        own_slab.append(np.ascontiguousarray(
            om.transpose(1, 0, 2).astype(np.float32)))
        dg = np.where(real, deg[snc], 0).astype(np.float32)
        deg_slab.append(np.ascontiguousarray(
            np.stack([dg, np.ones_like(dg)], axis=0)))
    return dict(GP=GP, GPC=GPC, e_slot=e_slot, slot_node=slot_node,
                tpos_slab=tpos_slab, upd_slab=upd_slab, own_slab=own_slab,
                deg_slab=deg_slab)


def _step_slabs(st, nodes_cur, edge_features, edge_source, edge_dest):
    GP = st["GP"]
    L = GP * CAP
    xcats, ndTs, ndUs = [], [], []
    for c in range(NCORES):
        es = st["e_slot"][c]
        ns = nodes_cur[edge_source[es]] + nodes_cur[edge_dest[es]]
        ef = edge_features[es]
        x1 = ns.reshape(L, 2, 128).transpose(2, 1, 0)
        x2 = ef.reshape(L, 2, 128).transpose(2, 1, 0)
        xc = np.concatenate([x1, x2], axis=1)          # [128, 4, L]
        xc = xc.reshape(128, 4, GP, CAP).transpose(0, 2, 1, 3)
        xcats.append(np.ascontiguousarray(xc))

        sn = st["slot_node"][c]
        nd = nodes_cur[np.maximum(sn, 0)] * (sn >= 0)[:, None]
        ndT = nd.reshape(GP, 128, 2, 128).transpose(3, 0, 2, 1)
        ndTs.append(np.ascontiguousarray(ndT))
        ndU = nd.reshape(GP, 128, 256).transpose(1, 0, 2)
        ndUs.append(np.ascontiguousarray(ndU))
    return xcats, ndTs, ndUs


def _step_weights(inp, p):
    wc = np.concatenate([inp["mn_W"][p], inp["mf_W"][p]], axis=1)  # [512, 512]
    ml2 = inp["ml2_W"][p].astype(np.float32)
    wih = inp["gru_Wih"][p].astype(np.float32)
    wfused = wih @ ml2                                  # [768, 512]
    bvec = wih @ inp["ml2_b"][p].astype(np.float32)     # [768]
    bih = inp["gru_bih"][p].astype(np.float32)
    bhh = inp["gru_bhh"][p].astype(np.float32)
    bhh2 = bhh.copy()
    bhh2[0:512] += bih[0:512]                           # fold r,z input bias
    rows = np.zeros((2, R_LEN), np.float32)
    rows[0, R_X:R_X + 512] = bvec[0:512]
    rows[1, R_X:R_X + 512] = bhh2[0:512]
    rows[0, R_Y:R_Y + 256] = bvec[512:768]
    rows[1, R_Y:R_Y + 256] = bih[512:768]
    rows[1, R_Z:R_Z + 256] = bhh2[512:768]
    rows[1, R_AGG + 0 * 512:R_AGG + 1 * 512] = inp["dec_t_b"]
    rows[1, R_AGG + 1 * 512:R_AGG + 2 * 512] = inp["dec_g_b"]
    rows[1, R_AGG + 2 * 512:R_AGG + 3 * 512] = inp["init_t_b"]
    rows[1, R_AGG + 3 * 512:R_AGG + 4 * 512] = inp["init_g_b"]
    return dict(
        wcat=_wT3(wc, 4),
        wfu=_wT3(wfused, 4),
        whh=_wT3(inp["gru_Whh"][p].astype(np.float32), 2),
        rows=rows,
    )


# ----------------------------------------------------------------------------
# Entry point
# ----------------------------------------------------------------------------

def kernel(**inputs):
    global last_exec_ns
    inp = {k: np.asarray(v) for k, v in inputs.items()}
    nodes0 = inp["nodes"].astype(np.float32)
    ef = inp["edge_features"].astype(np.float32)
    esrc = inp["edge_source"].astype(np.int64)
    edst = inp["edge_dest"].astype(np.int64)

    st = _structure(esrc, edst, inp["node_owner"].astype(np.int64),
                    inp["running"])
    GP = st["GP"]
    GPC = st["GPC"]

    trace = bool(os.environ.get("BASSK_TRACE"))
    if trace:
        _install_trace_hook()
    exec_ns = []

    def run(prog, maps):
        res = run_bass_kernel_spmd(prog, maps, list(range(NCORES)), trace=trace)
        exec_ns.append(res.exec_time_ns)
        return res.results

    # ---- launch A: step 0 ----
    if (GP, GPC, False) not in _progs:
        _progs[(GP, GPC, False)] = _build_prog(GP, GPC, False)
    w0 = _step_weights(inp, 0)
    xcats, ndTs, ndUs = _step_slabs(st, nodes0, ef, esrc, edst)
    maps = [dict(xcat=xcats[c], tpos=st["tpos_slab"][c], ndT=ndTs[c],
                 ndU=ndUs[c], upd=st["upd_slab"][c], degr=st["deg_slab"][c],
                 **w0)
            for c in range(NCORES)]
    resA = run(_progs[(GP, GPC, False)], maps)

    nodes1 = np.array(nodes0)
    for c in range(NCORES):
        sn = st["slot_node"][c]
        real = sn >= 0
        nodes1[sn[real]] = resA[c]["newnodes"][real]

    # ---- launch B: step 1 + aggregators ----
    if (GP, GPC, True) not in _progs:
        _progs[(GP, GPC, True)] = _build_prog(GP, GPC, True)
    w1 = _step_weights(inp, 1)
    wagg = np.ascontiguousarray(np.stack(
        [_wT3(inp["dec_t_W"].astype(np.float32), 2),
         _wT3(inp["dec_g_W"].astype(np.float32), 2),
         _wT3(inp["init_t_W"].astype(np.float32), 2),
         _wT3(inp["init_g_W"].astype(np.float32), 2)], axis=2))
    xcats, ndTs, ndUs = _step_slabs(st, nodes1, ef, esrc, edst)
    maps = [dict(xcat=xcats[c], tpos=st["tpos_slab"][c], ndT=ndTs[c],
                 ndU=ndUs[c], upd=st["upd_slab"][c], degr=st["deg_slab"][c],
                 omsk=st["own_slab"][c], wagg=wagg, **w1)
            for c in range(NCORES)]
    resB = run(_progs[(GP, GPC, True)], maps)

    pool_sum = np.zeros((128, 8, 512), np.float32)
    for c in range(NCORES):
        pool_sum += resB[c]["pool_out"]
    agg_dec = pool_sum[:, 0:4, :].transpose(1, 0, 2).reshape(B, A)
    agg_init = pool_sum[:, 4:8, :].transpose(1, 0, 2).reshape(B, A)

    # ---- tiny heads on host ----
    logits = agg_dec @ inp["ntd_W"].astype(np.float32).T \
        + inp["ntd_b"].astype(np.float32)
    sel = inp["ref_types"].astype(np.int64) + 1
    m = logits.max(axis=-1, keepdims=True)
    lse = m + np.log(np.exp(logits - m).sum(axis=-1, keepdims=True))
    logp = logits - lse
    per_ex = -logp[np.arange(B), sel]
    loss = np.where(np.asarray(inp["running"], bool), per_ex,
                    0.0).astype(np.float32).mean()
    emb = inp["nte"].astype(np.float32)[sel - 1]
    new_features = emb @ inp["f1_W"].astype(np.float32).T \
        + inp["f1_b"].astype(np.float32) \
        + agg_init @ inp["f2_W"].astype(np.float32).T

    if trace:
        last_exec_ns = exec_ns
    return (logits.astype(np.float32), new_features.astype(np.float32),
            np.float32(loss))


def _install_trace_hook():
    import sys
    import types
    if "antenv.axon_hooks" in sys.modules:
        return
    try:
        from trn_agent_boot.trn_boot import _ntff_profile_via_ctypes
        hook = _ntff_profile_via_ctypes("/opt/axon/libaxon_pjrt.so")
    except Exception:
        hook = None
    mod = types.ModuleType("antenv.axon_hooks")
    mod.get_axon_ntff_profile_hook = lambda: hook
    mod.set_axon_ntff_profile_hook = lambda h: None
    sys.modules["antenv.axon_hooks"] = mod


# revision 12
# speedup vs baseline: 1.1127x; 1.1126x over previous
"""Trainium2 Bass kernel for nn_NodeAdder (GGNN propagation + node-type head).

Strategy (8 NeuronCores, SPMD, no collectives):
  - Host bins the 65536 nodes into 8*GP groups of <=128 nodes such that each
    group receives <=512 scatter contributions (edge endpoints). All
    gather/scatter index structure is folded into host-prepared transposed DMA
    slabs; the device does dense fp32r matmuls, a banded mask-matmul segment
    reduction, and GRU pointwise math.
  - The message second layer is folded into the GRU input weights on the host
    (Wfused = Wih @ ml2W), so the device scatters tanh activations and the
    ml2 bias becomes a rank-1 deg x (Wih @ ml2_b) term that rides a K=1
    matmul. All other biases also ride K=1 matmuls straight into PSUM.
  - Launch A runs propagation step 0; host reassembles node state and
    regathers; launch B runs step 1 plus the two gated aggregators pooled
    per-graph with owner-mask matmuls (accumulated in PSUM across all
    groups). Tiny heads finish on host.
"""

import os
import numpy as np
from contextlib import ExitStack

import concourse.tile as tile
from concourse import bacc, mybir
from concourse.bass_utils import run_bass_kernel_spmd
from concourse.masks import make_identity

F32 = mybir.dt.float32
F32R = mybir.dt.float32r
AF = mybir.ActivationFunctionType

B, N, E, S, A, T = 512, 65536, 131072, 256, 512, 64
NCORES = 8
CAP = 512          # contribution slots per group (4 subchunks of 128)
NSUB = CAP // 128

# rows2 layout (K=2 bias matmuls against [deg; ones]): [2, 3072]
R_X = 0            # 512: row0 bvec rz, row1 bhh' rz
R_Y = 512          # 256: row0 bvec n, row1 bih n
R_Z = 768          # 256: row0 bhh n (K=1 with ones)
R_AGG = 1024       # 4*512 on row0: dec_t_b, dec_g_b, init_t_b, init_g_b
R_LEN = 3072

_progs = {}
last_exec_ns = None


# ----------------------------------------------------------------------------
# Device program
# ----------------------------------------------------------------------------

def _build_prog(GP, with_agg):
    nc = bacc.Bacc("TRN2", target_bir_lowering=False, debug=False,
                   num_devices=NCORES)

    xcat = nc.dram_tensor("xcat", [128, GP, 4, CAP], F32R, kind="ExternalInput").ap()
    tpos = nc.dram_tensor("tpos", [128, GP * NSUB], F32, kind="ExternalInput").ap()
    ndT = nc.dram_tensor("ndT", [128, GP, 2, 128], F32R, kind="ExternalInput").ap()
    ndU = nc.dram_tensor("ndU", [128, GP, 256], F32, kind="ExternalInput").ap()
    upd = nc.dram_tensor("upd", [128, GP], F32, kind="ExternalInput").ap()
    degr = nc.dram_tensor("degr", [2, GP * 128], F32R, kind="ExternalInput").ap()
    rows = nc.dram_tensor("rows", [2, R_LEN], F32R, kind="ExternalInput").ap()
    wcat = nc.dram_tensor("wcat", [128, 4, 512], F32R, kind="ExternalInput").ap()
    wfu = nc.dram_tensor("wfu", [128, 4, 768], F32R, kind="ExternalInput").ap()
    whh = nc.dram_tensor("whh", [128, 2, 768], F32R, kind="ExternalInput").ap()
    if with_agg:
        omsk = nc.dram_tensor("omsk", [128, GP, 4, 128], F32R,
                              kind="ExternalInput").ap()
        wagg = nc.dram_tensor("wagg", [128, 2, 4, 512], F32R,
                              kind="ExternalInput").ap()
        pool_out = nc.dram_tensor("pool_out", [128, 8, 512], F32,
                                  kind="ExternalOutput").ap()
        n2Td = nc.dram_tensor("n2Td", [128, GP, 2, 128], F32R).ap()
    else:
        newnodes = nc.dram_tensor("newnodes", [GP * 128, 256], F32,
                                  kind="ExternalOutput").ap()

    with tile.TileContext(nc) as tc, ExitStack() as ctx:
        consts = ctx.enter_context(tc.tile_pool(name="consts", bufs=1))
        ident = consts.tile([128, 128], F32)
        make_identity(nc, ident[:])
        iota0 = consts.tile([128, 128], F32)
        nc.gpsimd.iota(iota0[:], pattern=[[1, 128]], base=0, channel_multiplier=0,
                       allow_small_or_imprecise_dtypes=True)
        rows_t = consts.tile([2, R_LEN], F32R)
        nc.sync.dma_start(rows_t[:], rows[:])
        degr_t = consts.tile([2, GP * 128], F32R)
        nc.sync.dma_start(degr_t[:], degr[:])
        wcat_t = consts.tile([128, 4, 512], F32R)
        nc.sync.dma_start(wcat_t[:], wcat[:])
        wfu_t = consts.tile([128, 4, 768], F32R)
        nc.sync.dma_start(wfu_t[:], wfu[:])
        whh_t = consts.tile([128, 2, 768], F32R)
        nc.sync.dma_start(whh_t[:], whh[:])
        tpos_t = consts.tile([128, GP * NSUB], F32)
        nc.sync.dma_start(tpos_t[:], tpos[:])
        upd_t = consts.tile([128, GP], F32)
        nc.sync.dma_start(upd_t[:], upd[:])

        with ExitStack() as p1:
            slab_p = p1.enter_context(tc.tile_pool(name="slab", bufs=3))
            tpre_p = p1.enter_context(tc.tile_pool(name="tpre", bufs=2))
            mask_p = p1.enter_context(tc.tile_pool(name="mask", bufs=3))
            nd_p = p1.enter_context(tc.tile_pool(name="nd", bufs=3))
            gru_p = p1.enter_context(tc.tile_pool(name="gru", bufs=3))
            inpT_p = p1.enter_context(tc.tile_pool(name="inpT", bufs=2))
            out_p = p1.enter_context(tc.tile_pool(name="outp", bufs=3))
            pre_ps = p1.enter_context(tc.tile_pool(name="pre_ps", bufs=2,
                                                   space="PSUM"))
            scat_ps = p1.enter_context(tc.tile_pool(name="scat_ps", bufs=2,
                                                    space="PSUM"))
            x_ps = p1.enter_context(tc.tile_pool(name="x_ps", bufs=2,
                                                 space="PSUM"))
            yz_ps = p1.enter_context(tc.tile_pool(name="yz_ps", bufs=1,
                                                  space="PSUM"))

            for g in range(GP):
                slab = slab_p.tile([128, 4, CAP], F32R)
                nc.sync.dma_start(slab[:], xcat[:, g, :, :])

                # tanh(pre) per 128-contribution subchunk, token-major
                tpre = tpre_p.tile([128, 4, 512], F32R)
                scat = scat_ps.tile([128, 512], F32)
                for s in range(NSUB):
                    pp = pre_ps.tile([128, 512], F32)
                    for t in range(4):
                        nc.tensor.matmul(
                            pp[:], lhsT=slab[:, t, s * 128:(s + 1) * 128],
                            rhs=wcat_t[:, t, :], start=(t == 0), stop=(t == 3))
                    nc.scalar.activation(tpre[:, s, :], pp[:], AF.Tanh)
                    mask = mask_p.tile([128, 128], F32R)
                    nc.vector.tensor_tensor(
                        out=mask[:],
                        in0=tpos_t[:, g * NSUB + s:g * NSUB + s + 1]
                        .to_broadcast([128, 128]),
                        in1=iota0[:], op=mybir.AluOpType.is_equal)
                    nc.tensor.matmul(scat[:], lhsT=mask[:], rhs=tpre[:, s, :],
                                     start=(s == 0), stop=(s == NSUB - 1))

                # pooled tanh [node, 512] -> transposed inpT[p, t, m]
                inp_u = out_p.tile([128, 512], F32)
                nc.scalar.activation(inp_u[:], scat[:], AF.Copy)
                tp = x_ps.tile([128, 512], F32, tag="xps")
                for t in range(4):
                    nc.tensor.transpose(tp[:, t * 128:(t + 1) * 128],
                                        inp_u[:, t * 128:(t + 1) * 128], ident[:])
                inpT = inpT_p.tile([128, 4, 128], F32R)
                nc.vector.tensor_copy(
                    inpT[:], tp[:].rearrange("p (t n) -> p t n", t=4))

                ndT_g = nd_p.tile([128, 2, 128], F32R)
                nc.sync.dma_start(ndT_g[:], ndT[:, g, :, :])
                ndU_g = nd_p.tile([128, 256], F32)
                nc.sync.dma_start(ndU_g[:], ndU[:, g, :])

                dg = degr_t[:, g * 128:(g + 1) * 128]
                # X = (ir+hr | iz+hz), Y = inn, Z = hn -- biases via K=2 mm
                X = x_ps.tile([128, 512], F32, tag="xps")
                nc.tensor.matmul(X[:], lhsT=dg, rhs=rows_t[:, R_X:R_X + 512],
                                 start=True, stop=False)
                for t in range(4):
                    nc.tensor.matmul(X[:], lhsT=inpT[:, t, :],
                                     rhs=wfu_t[:, t, 0:512],
                                     start=False, stop=False)
                for t in range(2):
                    nc.tensor.matmul(X[:], lhsT=ndT_g[:, t, :],
                                     rhs=whh_t[:, t, 0:512],
                                     start=False, stop=(t == 1))
                Y = yz_ps.tile([128, 256], F32, tag="y")
                nc.tensor.matmul(Y[:], lhsT=dg, rhs=rows_t[:, R_Y:R_Y + 256],
                                 start=True, stop=False)
                for t in range(4):
                    nc.tensor.matmul(Y[:], lhsT=inpT[:, t, :],
                                     rhs=wfu_t[:, t, 512:768],
                                     start=False, stop=(t == 3))
                Z = yz_ps.tile([128, 256], F32, tag="z")
                nc.tensor.matmul(Z[:], lhsT=dg, rhs=rows_t[:, R_Z:R_Z + 256],
                                 start=True, stop=False)
                for t in range(2):
                    nc.tensor.matmul(Z[:], lhsT=ndT_g[:, t, :],
                                     rhs=whh_t[:, t, 512:768],
                                     start=False, stop=(t == 1))

                r = gru_p.tile([128, 256], F32)
                nc.scalar.activation(r[:], X[:, 0:256], AF.Sigmoid)
                zp = gru_p.tile([128, 256], F32)   # 1 - z = sigmoid(-(iz+hz))
                nc.scalar.activation(zp[:], X[:, 256:512], AF.Sigmoid, scale=-1.0)
                tn = gru_p.tile([128, 256], F32)
                nc.vector.tensor_mul(tn[:], r[:], Z[:])
                nc.vector.tensor_add(tn[:], tn[:], Y[:])
                nn = gru_p.tile([128, 256], F32)
                nc.scalar.activation(nn[:], tn[:], AF.Tanh)
                um = gru_p.tile([128, 256], F32)
                nc.vector.tensor_tensor(
                    out=um[:], in0=zp[:],
                    in1=upd_t[:, g:g + 1].to_broadcast([128, 256]),
                    op=mybir.AluOpType.mult)
                dnh = gru_p.tile([128, 256], F32)
                nc.vector.tensor_sub(dnh[:], nn[:], ndU_g[:])
                nc.vector.tensor_mul(dnh[:], dnh[:], um[:])
                newn = out_p.tile([128, 256], F32)
                nc.vector.tensor_add(newn[:], ndU_g[:], dnh[:])

                if with_agg:
                    n2ps = yz_ps.tile([128, 256], F32, tag="y")
                    for t in range(2):
                        nc.tensor.transpose(n2ps[:, t * 128:(t + 1) * 128],
                                            newn[:, t * 128:(t + 1) * 128],
                                            ident[:])
                    n2T = inpT_p.tile([128, 2, 128], F32R)
                    nc.vector.tensor_copy(
                        n2T[:], n2ps[:].rearrange("p (t n) -> p t n", t=2))
                    nc.sync.dma_start(n2Td[:, g, :, :], n2T[:])
                else:
                    nc.sync.dma_start(newnodes[g * 128:(g + 1) * 128, :], newn[:])

        if with_agg:
            with ExitStack() as p2:
                agg_c = p2.enter_context(tc.tile_pool(name="agg_c", bufs=1))
                wagg_t = agg_c.tile([128, 2, 4, 512], F32R)
                nc.sync.dma_start(wagg_t[:], wagg[:])
                pooled_sb = agg_c.tile([128, 8, 512], F32)

                a_sb = p2.enter_context(tc.tile_pool(name="a_sb", bufs=3))
                a_m = p2.enter_context(tc.tile_pool(name="a_m", bufs=3))
                a_ps = p2.enter_context(tc.tile_pool(name="a_ps", bufs=4,
                                                     space="PSUM"))
                p_ps = p2.enter_context(tc.tile_pool(name="p_ps", bufs=1,
                                                     space="PSUM"))
                for a in range(2):
                    pq = [p_ps.tile([128, 512], F32, tag=f"pot{ot}",
                                    name=f"pq{a}_{ot}")
                          for ot in range(4)]
                    for g in range(GP):
                        n2g = a_sb.tile([128, 2, 128], F32R)
                        nc.sync.dma_start(n2g[:], n2Td[:, g, :, :])
                        omask = a_m.tile([128, 4, 128], F32R)
                        nc.sync.dma_start(omask[:], omsk[:, g, :, :])
                        dp = a_ps.tile([128, 512], F32, tag="aps")
                        for t in range(2):
                            nc.tensor.matmul(dp[:], lhsT=n2g[:, t, :],
                                             rhs=wagg_t[:, t, 2 * a, :],
                                             start=(t == 0), stop=False)
                        nc.tensor.matmul(
                            dp[:], lhsT=degr_t[:, 0:128],
                            rhs=rows_t[:, R_AGG + (2 * a) * 512:
                                       R_AGG + (2 * a + 1) * 512],
                            start=False, stop=True)
                        gp2 = a_ps.tile([128, 512], F32, tag="aps")
                        for t in range(2):
                            nc.tensor.matmul(gp2[:], lhsT=n2g[:, t, :],
                                             rhs=wagg_t[:, t, 2 * a + 1, :],
                                             start=(t == 0), stop=False)
                        nc.tensor.matmul(
                            gp2[:], lhsT=degr_t[:, 0:128],
                            rhs=rows_t[:, R_AGG + (2 * a + 1) * 512:
                                       R_AGG + (2 * a + 2) * 512],
                            start=False, stop=True)
                        gates = a_sb.tile([128, 512], F32)
                        nc.scalar.activation(gates[:], gp2[:], AF.Sigmoid)
                        gated = a_sb.tile([128, 512], F32R)
                        nc.vector.tensor_mul(gated[:], dp[:], gates[:])
                        for ot in range(4):
                            nc.tensor.matmul(pq[ot][:], lhsT=omask[:, ot, :],
                                             rhs=gated[:], start=(g == 0),
                                             stop=(g == GP - 1))
                    for ot in range(4):
                        nc.vector.tensor_copy(pooled_sb[:, 4 * a + ot, :],
                                              pq[ot][:])
                nc.sync.dma_start(pool_out[:], pooled_sb[:])

    nc.compile()
    return nc


# ----------------------------------------------------------------------------
# Host-side index structure and slab packing
# ----------------------------------------------------------------------------

def _wT3(W, kt):
    # W [fout, fin] -> [128, kt, fout] with [p, t, f] = W[f, 128*t + p]
    fout = W.shape[0]
    return np.ascontiguousarray(
        W.T.reshape(kt, 128, fout).transpose(1, 0, 2)).astype(np.float32)


def _structure(edge_source, edge_dest, node_owner, running):
    deg = (np.bincount(edge_source, minlength=N)
           + np.bincount(edge_dest, minlength=N)).astype(np.int64)
    assert deg.max() <= CAP
    bin_id = np.empty(N, np.int32)
    pos = np.empty(N, np.int32)
    b = 0
    cnt = 0
    csum = 0
    degl = deg.tolist()
    for n in range(N):
        d = degl[n]
        if cnt >= 128 or csum + d > CAP:
            b += 1
            cnt = 0
            csum = 0
        bin_id[n] = b
        pos[n] = cnt
        cnt += 1
        csum += d
    nbins = b + 1
    GP = -(-nbins // NCORES)

    tgt = np.concatenate([edge_dest, edge_source])
    eid = np.concatenate([np.arange(E, dtype=np.int64)] * 2)
    tb = bin_id[tgt]
    order = np.argsort(tb, kind="stable")
    tb_s = tb[order]
    eid_s = eid[order]
    tpos_s = pos[tgt][order].astype(np.float32)
    counts = np.bincount(tb_s, minlength=NCORES * GP)
    assert counts.max() <= CAP
    starts = np.concatenate([[0], np.cumsum(counts)])

    L = GP * CAP
    e_slot = np.zeros((NCORES, L), np.int64)
    tp_slot = np.full((NCORES, L), 300.0, np.float32)
    for bb in range(nbins):
        c, g = divmod(bb, GP)
        s0 = int(starts[bb])
        n = int(counts[bb])
        e_slot[c, g * CAP:g * CAP + n] = eid_s[s0:s0 + n]
        tp_slot[c, g * CAP:g * CAP + n] = tpos_s[s0:s0 + n]

    slot_node = np.full((NCORES, GP * 128), -1, np.int64)
    core_of = bin_id // GP
    slot_of = (bin_id % GP) * 128 + pos
    slot_node[core_of, slot_of] = np.arange(N)

    run_f = np.asarray(running, bool)
    tpos_slab, upd_slab, own_slab, deg_slab = [], [], [], []
    for c in range(NCORES):
        tpos_slab.append(np.ascontiguousarray(
            tp_slot[c].reshape(GP, NSUB, 128).transpose(2, 0, 1)
            .reshape(128, GP * NSUB)))
        sn = slot_node[c]
        real = sn >= 0
        snc = np.maximum(sn, 0)
        u = (run_f[node_owner[snc]] & real).astype(np.float32)
        upd_slab.append(np.ascontiguousarray(u.reshape(GP, 128).T))
        ow = np.where(real, node_owner[snc], 10000)
        om = (ow[:, None] == np.arange(B)[None, :]).astype(np.float32)
        own_slab.append(np.ascontiguousarray(
            om.reshape(GP, 128, 4, 128).transpose(1, 0, 2, 3)))
        dg = np.where(real, deg[snc], 0).astype(np.float32)
        deg_slab.append(np.ascontiguousarray(
            np.stack([dg, np.ones_like(dg)], axis=0)))
    return dict(GP=GP, e_slot=e_slot, slot_node=slot_node,
                tpos_slab=tpos_slab, upd_slab=upd_slab, own_slab=own_slab,
                deg_slab=deg_slab)


def _step_slabs(st, nodes_cur, edge_features, edge_source, edge_dest):
    GP = st["GP"]
    L = GP * CAP
    xcats, ndTs, ndUs = [], [], []
    for c in range(NCORES):
        es = st["e_slot"][c]
        ns = nodes_cur[edge_source[es]] + nodes_cur[edge_dest[es]]
        ef = edge_features[es]
        x1 = ns.reshape(L, 2, 128).transpose(2, 1, 0)
        x2 = ef.reshape(L, 2, 128).transpose(2, 1, 0)
        xc = np.concatenate([x1, x2], axis=1)          # [128, 4, L]
        xc = xc.reshape(128, 4, GP, CAP).transpose(0, 2, 1, 3)
        xcats.append(np.ascontiguousarray(xc))

        sn = st["slot_node"][c]
        nd = nodes_cur[np.maximum(sn, 0)] * (sn >= 0)[:, None]
        ndT = nd.reshape(GP, 128, 2, 128).transpose(3, 0, 2, 1)
        ndTs.append(np.ascontiguousarray(ndT))
        ndU = nd.reshape(GP, 128, 256).transpose(1, 0, 2)
        ndUs.append(np.ascontiguousarray(ndU))
    return xcats, ndTs, ndUs


def _step_weights(inp, p):
    wc = np.concatenate([inp["mn_W"][p], inp["mf_W"][p]], axis=1)  # [512, 512]
    ml2 = inp["ml2_W"][p].astype(np.float32)
    wih = inp["gru_Wih"][p].astype(np.float32)
    wfused = wih @ ml2                                  # [768, 512]
    bvec = wih @ inp["ml2_b"][p].astype(np.float32)     # [768]
    bih = inp["gru_bih"][p].astype(np.float32)
    bhh = inp["gru_bhh"][p].astype(np.float32)
    bhh2 = bhh.copy()
    bhh2[0:512] += bih[0:512]                           # fold r,z input bias
    rows = np.zeros((2, R_LEN), np.float32)
    rows[0, R_X:R_X + 512] = bvec[0:512]
    rows[1, R_X:R_X + 512] = bhh2[0:512]
    rows[0, R_Y:R_Y + 256] = bvec[512:768]
    rows[1, R_Y:R_Y + 256] = bih[512:768]
    rows[1, R_Z:R_Z + 256] = bhh2[512:768]
    rows[1, R_AGG + 0 * 512:R_AGG + 1 * 512] = inp["dec_t_b"]
    rows[1, R_AGG + 1 * 512:R_AGG + 2 * 512] = inp["dec_g_b"]
    rows[1, R_AGG + 2 * 512:R_AGG + 3 * 512] = inp["init_t_b"]
    rows[1, R_AGG + 3 * 512:R_AGG + 4 * 512] = inp["init_g_b"]
    return dict(
        wcat=_wT3(wc, 4),
        wfu=_wT3(wfused, 4),
        whh=_wT3(inp["gru_Whh"][p].astype(np.float32), 2),
        rows=rows,
    )


# ----------------------------------------------------------------------------
# Entry point
# ----------------------------------------------------------------------------

def kernel(**inputs):
    global last_exec_ns
    inp = {k: np.asarray(v) for k, v in inputs.items()}
    nodes0 = inp["nodes"].astype(np.float32)
    ef = inp["edge_features"].astype(np.float32)
    esrc = inp["edge_source"].astype(np.int64)
    edst = inp["edge_dest"].astype(np.int64)

    st = _structure(esrc, edst, inp["node_owner"].astype(np.int64),
                    inp["running"])
    GP = st["GP"]

    trace = bool(os.environ.get("BASSK_TRACE"))
    if trace:
        _install_trace_hook()
    exec_ns = []

    def run(prog, maps):
        res = run_bass_kernel_spmd(prog, maps, list(range(NCORES)), trace=trace)
        exec_ns.append(res.exec_time_ns)
        return res.results

    # ---- launch A: step 0 ----
    if (GP, False) not in _progs:
        _progs[(GP, False)] = _build_prog(GP, False)
    w0 = _step_weights(inp, 0)
    xcats, ndTs, ndUs = _step_slabs(st, nodes0, ef, esrc, edst)
    maps = [dict(xcat=xcats[c], tpos=st["tpos_slab"][c], ndT=ndTs[c],
                 ndU=ndUs[c], upd=st["upd_slab"][c], degr=st["deg_slab"][c],
                 **w0)
            for c in range(NCORES)]
    resA = run(_progs[(GP, False)], maps)

    nodes1 = np.array(nodes0)
    for c in range(NCORES):
        sn = st["slot_node"][c]
        real = sn >= 0
        nodes1[sn[real]] = resA[c]["newnodes"][real]

    # ---- launch B: step 1 + aggregators ----
    if (GP, True) not in _progs:
        _progs[(GP, True)] = _build_prog(GP, True)
    w1 = _step_weights(inp, 1)
    wagg = np.ascontiguousarray(np.stack(
        [_wT3(inp["dec_t_W"].astype(np.float32), 2),
         _wT3(inp["dec_g_W"].astype(np.float32), 2),
         _wT3(inp["init_t_W"].astype(np.float32), 2),
         _wT3(inp["init_g_W"].astype(np.float32), 2)], axis=2))
    xcats, ndTs, ndUs = _step_slabs(st, nodes1, ef, esrc, edst)
    maps = [dict(xcat=xcats[c], tpos=st["tpos_slab"][c], ndT=ndTs[c],
                 ndU=ndUs[c], upd=st["upd_slab"][c], degr=st["deg_slab"][c],
                 omsk=st["own_slab"][c], wagg=wagg, **w1)
            for c in range(NCORES)]
    resB = run(_progs[(GP, True)], maps)

    pool_sum = np.zeros((128, 8, 512), np.float32)
    for c in range(NCORES):
        pool_sum += resB[c]["pool_out"]
    agg_dec = pool_sum[:, 0:4, :].transpose(1, 0, 2).reshape(B, A)
    agg_init = pool_sum[:, 4:8, :].transpose(1, 0, 2).reshape(B, A)

    # ---- tiny heads on host ----
    logits = agg_dec @ inp["ntd_W"].astype(np.float32).T \
        + inp["ntd_b"].astype(np.float32)
    sel = inp["ref_types"].astype(np.int64) + 1
    m = logits.max(axis=-1, keepdims=True)
    lse = m + np.log(np.exp(logits - m).sum(axis=-1, keepdims=True))
    logp = logits - lse
    per_ex = -logp[np.arange(B), sel]
    loss = np.where(np.asarray(inp["running"], bool), per_ex,
                    0.0).astype(np.float32).mean()
    emb = inp["nte"].astype(np.float32)[sel - 1]
    new_features = emb @ inp["f1_W"].astype(np.float32).T \
        + inp["f1_b"].astype(np.float32) \
        + agg_init @ inp["f2_W"].astype(np.float32).T

    if trace:
        last_exec_ns = exec_ns
    return (logits.astype(np.float32), new_features.astype(np.float32),
            np.float32(loss))


def _install_trace_hook():
    import sys
    import types
    if "antenv.axon_hooks" in sys.modules:
        return
    try:
        from trn_agent_boot.trn_boot import _ntff_profile_via_ctypes
        hook = _ntff_profile_via_ctypes("/opt/axon/libaxon_pjrt.so")
    except Exception:
        hook = None
    mod = types.ModuleType("antenv.axon_hooks")
    mod.get_axon_ntff_profile_hook = lambda: hook
    mod.set_axon_ntff_profile_hook = lambda h: None
    sys.modules["antenv.axon_hooks"] = mod
